# revision 4
# baseline (speedup 1.0000x reference)
"""MAE ViT encoder (nn_MaskedAutoencoderViT) Trainium2 Bass kernel.

Strategy: data-parallel over batch (16 images -> 8 cores x 2 images).
Feature-major activation layout on chip: activations stored transposed as
[128 partitions (d chunk), 6 chunks, 152 tokens] so every matmul is
weight-stationary (lhsT = 128x128 weight tile, rhs = activation columns)
with zero on-device transposes.  Attention is computed in transposed form
(S^T = (K^T)-stationary @ Q^T), softmax uses the structure
exp(att)/ (sum + 1e-9) (the reference's global-max subtraction cancels in
the normalization up to ~1e-10 relative, far below fp32 noise).
Matmul operands in fp16 (full PE rate, 11-bit mantissa), accumulation and
residual stream in fp32.

Host side does only data marshalling: noise argsort, patch gather,
pos-embed gathers, weight transposition + fp16 cast.
"""
import numpy as np
from contextlib import ExitStack

import concourse.bass as bass
import concourse.bacc as bacc
import concourse.mybir as mybir
import concourse.tile as tile
from concourse.bass_utils import run_bass_kernel_spmd

F16 = mybir.dt.float16
F32 = mybir.dt.float32
AF = mybir.ActivationFunctionType
OP = mybir.AluOpType

# --- model config (hardcoded from the problem spec) ---
B, C_IN, H_IN, W_IN = 16, 1, 12, 2500
P_, Q_ = 1, 100
D, NH, DEPTH = 768, 12, 12
GH, GW = 12, 25
L = GH * GW                      # 300
LEN_KEEP = 75
HD = D // NH                     # 64
SCALE = HD ** -0.5               # 0.125
EPS_LN = 1e-5
MLP = 4 * D                      # 3072

NCORES = 8
BL = B // NCORES                 # 2 images per core
KT = 1 + LEN_KEEP                # 76 tokens per image
T = BL * KT                      # 152 token columns per core
NCH = D // 128                   # 6 feature chunks
MCH = MLP // 128                 # 24 mlp chunks
PIX = P_ * Q_                    # 100 pixels per patch


def bfree(ap, n, at=1):
    """Insert a 0-step (broadcast) free dim of size n at position `at`."""
    new_ap = list(ap.ap[:at]) + [[0, n]] + list(ap.ap[at:])
    return bass.AP(tensor=ap.tensor, offset=ap.offset, ap=new_ap)


def build(depth=DEPTH):
    nc = bacc.Bacc("TRN2", target_bir_lowering=False, debug=False,
                   num_devices=NCORES)

    # DRAM I/O
    patchesT = nc.dram_tensor("patchesT", [PIX, T], F16, kind="ExternalInput").ap()
    posT = nc.dram_tensor("posT", [NCH, 128, T], F32, kind="ExternalInput").ap()
    mvec = nc.dram_tensor("mvec", [BL, KT], F16, kind="ExternalInput").ap()
    wpatchT = nc.dram_tensor("wpatchT", [PIX, D], F16, kind="ExternalInput").ap()
    wqkvT = nc.dram_tensor("wqkvT", [depth, D, 3 * D], F16, kind="ExternalInput").ap()
    wprojT = nc.dram_tensor("wprojT", [depth, D, D], F16, kind="ExternalInput").ap()
    wfc1T = nc.dram_tensor("wfc1T", [depth, D, MLP], F16, kind="ExternalInput").ap()
    wfc2T = nc.dram_tensor("wfc2T", [depth, MLP, D], F16, kind="ExternalInput").ap()
    out_d = nc.dram_tensor("out", [BL, KT, D], F32, kind="ExternalOutput").ap()

    with tile.TileContext(nc) as tc, ExitStack() as ctx:
        pool = lambda name, bufs, **kw: ctx.enter_context(
            tc.tile_pool(name=name, bufs=bufs, **kw))

        const = pool("const", 1)
        hp = pool("hp", 1)
        lnp = pool("lnp", 1)
        yp = pool("yp", 2)
        tmpp = pool("tmpp", 1)
        qkp = pool("qkp", 1)
        vp = pool("vp", 2)
        ep = pool("ep", 2)
        pp = pool("pp", 2)
        otp = pool("otp", 1)
        gp = pool("gp", 1)
        bcp = pool("bcp", 4)
        tinyp = pool("tinyp", 8)
        medp = pool("medp", 3)
        wqkvp = pool("wqkvp", 7)
        wprojp = pool("wprojp", 9)
        wfc1p = pool("wfc1p", 7)
        wfc2p = pool("wfc2p", 24)

        psA = pool("psA", 3, space="PSUM")
        psB = pool("psB", 2, space="PSUM")
        psC = pool("psC", 3, space="PSUM")

        # constants
        ones16 = const.tile([128, 1], F16)
        nc.vector.memset(ones16[:], 1.0)
        eps_t = const.tile([1, 1], F32)
        nc.vector.memset(eps_t[:], EPS_LN)

        # static inputs
        patches_sb = const.tile([PIX, T], F16)
        nc.sync.dma_start(out=patches_sb[:], in_=patchesT[:])
        wpatch_sb = const.tile([PIX, D], F16)
        nc.sync.dma_start(out=wpatch_sb[:], in_=wpatchT[:])
        pos_sb = const.tile([128, NCH, T], F32)
        nc.sync.dma_start(out=pos_sb[:], in_=posT.rearrange("c p t -> p c t"))
        m_sb = const.tile([KT, BL], F16)
        nc.sync.dma_start(out=m_sb[:], in_=mvec.rearrange("b t -> t b"))

        # residual stream, feature-major fp32
        H = hp.tile([128, NCH, T], F32)

        # ---- patch embed + pos add ----
        for c in range(NCH):
            ps = psA.tile([128, T], F32, tag="psA")
            nc.tensor.matmul(ps[:], wpatch_sb[:, 128 * c:128 * (c + 1)],
                             patches_sb[:], start=True, stop=True)
            nc.vector.tensor_add(H[:, c, :], ps[:], pos_sb[:, c, :])

        def layernorm(src, out_dt, y_pool):
            """src: [128, NCH, T] fp32 -> returns normalized tile in out_dt."""
            lnin = lnp.tile([128, 2, NCH, T], F16, tag="lnin")
            nc.vector.tensor_copy(lnin[:, 0, :, :], src[:, :, :])
            nc.scalar.activation(lnin[:, 1, :, :], src[:, :, :], AF.Square)
            st = psC.tile([1, 2, T], F32, tag="psC")
            for c in range(NCH):
                nc.tensor.matmul(st[:], ones16[:, 0:1], lnin[:, :, c, :],
                                 start=(c == 0), stop=(c == NCH - 1))
            mean = tinyp.tile([1, T], F32, tag="tiny")
            ex2 = tinyp.tile([1, T], F32, tag="tiny")
            nc.vector.tensor_scalar_mul(mean[:], st[0:1, 0, :], 1.0 / D)
            nc.vector.tensor_scalar_mul(ex2[:], st[0:1, 1, :], 1.0 / D)
            var = tinyp.tile([1, T], F32, tag="tiny")
            nc.vector.scalar_tensor_tensor(var[:], mean[:], -1.0, mean[:],
                                           op0=OP.mult, op1=OP.mult)
            nc.vector.tensor_add(var[:], var[:], ex2[:])  # E[x^2] - mean^2
            nc.scalar.activation(var[:], var[:], AF.Ln, bias=eps_t[0:1, 0:1])
            rstd = tinyp.tile([1, T], F32, tag="tiny")
            nc.scalar.activation(rstd[:], var[:], AF.Exp, scale=-0.5)
            nb = tinyp.tile([1, T], F32, tag="tiny")
            nc.vector.scalar_tensor_tensor(nb[:], mean[:], -1.0, rstd[:],
                                           op0=OP.mult, op1=OP.mult)
            ab_b = bcp.tile([128, T], F32, tag="bc")
            nb_b = bcp.tile([128, T], F32, tag="bc")
            nc.gpsimd.partition_broadcast(ab_b[:], rstd[:])
            nc.gpsimd.partition_broadcast(nb_b[:], nb[:])
            tmp = tmpp.tile([128, NCH, T], F32, tag="tmp")
            nc.vector.tensor_mul(tmp[:], src[:, :, :], bfree(ab_b[:], NCH))
            y = y_pool.tile([128, NCH, T], out_dt, tag=f"y{out_dt}")
            nc.vector.tensor_add(y[:], tmp[:], bfree(nb_b[:], NCH))
            return y

        for l in range(depth):
            # weight loads for this layer (emitted first so DMA starts early)
            wqkv = [wqkvp.tile([128, 3 * D], F16, tag="wqkv", name="wqkv") for _ in range(NCH)]
            for k in range(NCH):
                nc.sync.dma_start(out=wqkv[k][:], in_=wqkvT[l, 128 * k:128 * (k + 1), :])
            wproj = [wprojp.tile([128, D], F16, tag="wproj", name="wproj") for _ in range(NCH)]
            for k in range(NCH):
                nc.sync.dma_start(out=wproj[k][:], in_=wprojT[l, 128 * k:128 * (k + 1), :])
            wfc1 = [wfc1p.tile([128, MLP], F16, tag="wfc1", name="wfc1") for _ in range(NCH)]
            for k in range(NCH):
                nc.sync.dma_start(out=wfc1[k][:], in_=wfc1T[l, 128 * k:128 * (k + 1), :])
            wfc2 = [wfc2p.tile([128, D], F16, tag="wfc2", name="wfc2") for _ in range(MCH)]
            for k in range(MCH):
                nc.sync.dma_start(out=wfc2[k][:], in_=wfc2T[l, 128 * k:128 * (k + 1), :])

            # ---- LN1 ----
            y1 = layernorm(H, F16, yp)

            # ---- QKV: Q,K feature-major ----
            qk16 = qkp.tile([128, 2 * NCH, T], F16, tag="qk")
            for oc in range(2 * NCH):
                ps = psA.tile([128, T], F32, tag="psA")
                for k in range(NCH):
                    nc.tensor.matmul(ps[:], wqkv[k][:, 128 * oc:128 * (oc + 1)],
                                     y1[:, k, :], start=(k == 0), stop=(k == NCH - 1))
                nc.vector.tensor_copy(qk16[:, oc, :], ps[:])

            # ---- V token-major per image ----
            v16 = []
            for b in range(BL):
                vps0 = psC.tile([KT, 512], F32, tag="psC")
                vps1 = psC.tile([KT, 512], F32, tag="psC")
                for k in range(NCH):
                    nc.tensor.matmul(vps0[:, 0:512],
                                     y1[:, k, KT * b:KT * (b + 1)],
                                     wqkv[k][:, 2 * D:2 * D + 512],
                                     start=(k == 0), stop=(k == NCH - 1))
                for k in range(NCH):
                    nc.tensor.matmul(vps1[:, 0:256],
                                     y1[:, k, KT * b:KT * (b + 1)],
                                     wqkv[k][:, 2 * D + 512:3 * D],
                                     start=(k == 0), stop=(k == NCH - 1))
                v = vp.tile([KT, D], F16, tag="v")
                nc.vector.tensor_copy(v[:, 0:512], vps0[:, 0:512])
                nc.vector.tensor_copy(v[:, 512:768], vps1[:, 0:256])
                v16.append(v)

            # ---- attention per image; heads grouped by parity ----
            ot16 = otp.tile([128, NCH, T], F16, tag="ot")
            for b in range(BL):
                e16 = ep.tile([KT, 2, 6 * KT], F16, tag="e")
                rs = []
                for g in range(2):
                    sps = psC.tile([KT, 512], F32, tag="psC")
                    for j in range(6):
                        nc.tensor.matmul(
                            sps[:, KT * j:KT * (j + 1)],
                            qk16[64 * g:64 * (g + 1), 6 + j, KT * b:KT * (b + 1)],
                            qk16[64 * g:64 * (g + 1), j, KT * b:KT * (b + 1)],
                            start=True, stop=True)
                    nc.scalar.activation(e16[:, g, :], sps[:, 0:6 * KT],
                                         AF.Exp, scale=SCALE)
                # row sums (masked): lhsT = m vector -> [1, 6*KT] per group
                recip = medp.tile([1, 2, 6 * KT], F32, tag="med")
                for g in range(2):
                    rps = psC.tile([1, 512], F32, tag="psC")
                    nc.tensor.matmul(rps[0:1, 0:6 * KT], m_sb[:, b:b + 1],
                                     e16[:, g, :], start=True, stop=True)
                    nc.vector.tensor_scalar_add(rps[0:1, 0:6 * KT],
                                                rps[0:1, 0:6 * KT], 1e-9)
                    nc.vector.reciprocal(recip[0:1, g, :], rps[0:1, 0:6 * KT])
                rc16 = medp.tile([1, 2, 6 * KT], F16, tag="med")
                nc.vector.tensor_copy(rc16[:], recip[:])
                rb = bcp.tile([KT, 2, 6 * KT], F16, tag="rb")
                nc.gpsimd.partition_broadcast(rb[:], rc16[:])
                p16 = pp.tile([KT, 2, 6 * KT], F16, tag="p")
                nc.vector.scalar_tensor_tensor(p16[:], e16[:, :, :],
                                               m_sb[:, b:b + 1], rb[:, :, :],
                                               op0=OP.mult, op1=OP.mult)
                for g in range(2):
                    ops = psC.tile([64, 512], F32, tag="psC")
                    for j in range(6):
                        nc.tensor.matmul(
                            ops[:, KT * j:KT * (j + 1)],
                            v16[b][:, 128 * j + 64 * g:128 * j + 64 * g + 64],
                            p16[:, g, KT * j:KT * (j + 1)],
                            start=True, stop=True)
                    nc.vector.tensor_copy(
                        ot16[64 * g:64 * (g + 1), :, KT * b:KT * (b + 1)],
                        ops[:, 0:6 * KT].rearrange("p (j t) -> p j t", j=6))

            # ---- proj + residual ----
            for oc in range(NCH):
                ps = psA.tile([128, T], F32, tag="psA")
                for k in range(NCH):
                    nc.tensor.matmul(ps[:], wproj[k][:, 128 * oc:128 * (oc + 1)],
                                     ot16[:, k, :], start=(k == 0), stop=(k == NCH - 1))
                nc.vector.tensor_add(H[:, oc, :], H[:, oc, :], ps[:])

            # ---- LN2 + MLP ----
            y2 = layernorm(H, F16, yp)
            g16 = gp.tile([128, MCH, T], F16, tag="g")
            for grp in range(MCH // 3):
                ps3 = psB.tile([128, 3, T], F32, tag="psB")
                for i in range(3):
                    oc = 3 * grp + i
                    for k in range(NCH):
                        nc.tensor.matmul(ps3[:, i, :],
                                         wfc1[k][:, 128 * oc:128 * (oc + 1)],
                                         y2[:, k, :], start=(k == 0), stop=(k == NCH - 1))
                nc.scalar.activation(g16[:, 3 * grp:3 * (grp + 1), :], ps3[:, :, :],
                                     AF.Gelu)
            for oc in range(NCH):
                ps = psA.tile([128, T], F32, tag="psA")
                for k in range(MCH):
                    nc.tensor.matmul(ps[:], wfc2[k][:, 128 * oc:128 * (oc + 1)],
                                     g16[:, k, :], start=(k == 0), stop=(k == MCH - 1))
                nc.vector.tensor_add(H[:, oc, :], H[:, oc, :], ps[:])

        # ---- final LN (fp32 out) + store ----
        yf = layernorm(H, F32, yp)
        o_r = out_d.rearrange("b t (c p) -> p c (b t)", p=128)
        for c in range(NCH):
            nc.sync.dma_start(out=o_r[:, c, :], in_=yf[:, c, :])

    nc.compile()
    return nc


def prep_inputs(inputs, depth=DEPTH):
    """Host-side marshalling. Returns per-core in_maps list."""
    g = {k: np.asarray(v) for k, v in inputs.items()}
    x = g["x"].astype(np.float32)
    noise = g["noise"].astype(np.float32)
    attn_mask = g["attn_mask"].astype(np.float32)
    ids_y = g["pos_embed_y_ids"].astype(np.int64)

    ids_shuffle = np.argsort(noise, axis=1, kind="stable")
    ids_keep = ids_shuffle[:, :LEN_KEEP]                      # (B, 75)

    patches = x.reshape(B, GH, GW, Q_).reshape(B, L, Q_)      # (B, 300, 100)
    mask_l = attn_mask.reshape(B, L)

    # pos vector per patch: [pos_y(384) | pos_x(384) * mask]
    pos_y = g["pos_y_table"].astype(np.float32)               # (13, 384)
    pos_x = g["pos_embed_x"].astype(np.float32)[0]            # (26, 384)
    ids_y_l = ids_y.reshape(B, L)
    gw_idx = np.tile(np.arange(GW), GH)                       # (300,)
    pos_full = np.zeros((B, L, D), np.float32)
    pos_full[:, :, :D // 2] = pos_y[ids_y_l]
    pos_full[:, :, D // 2:] = mask_l[:, :, None] * pos_x[gw_idx + 1][None]

    cls_vec = g["cls_token"].astype(np.float32).reshape(D).copy()
    cls_vec[D // 2:] += pos_x[0]

    wqkvT = np.ascontiguousarray(
        g["qkv_w"].astype(np.float32).transpose(0, 2, 1)[:depth]).astype(np.float16)
    wprojT = np.ascontiguousarray(
        g["proj_w"].astype(np.float32).transpose(0, 2, 1)[:depth]).astype(np.float16)
    wfc1T = np.ascontiguousarray(
        g["fc1_w"].astype(np.float32).transpose(0, 2, 1)[:depth]).astype(np.float16)
    wfc2T = np.ascontiguousarray(
        g["fc2_w"].astype(np.float32).transpose(0, 2, 1)[:depth]).astype(np.float16)
    wpatchT = np.ascontiguousarray(
        g["conv_w"].astype(np.float32).reshape(D, Q_).T).astype(np.float16)

    in_maps = []
    for core in range(NCORES):
        patchesT = np.zeros((PIX, T), np.float16)
        posT = np.zeros((D, T), np.float32)
        mv = np.zeros((BL, KT), np.float16)
        for b in range(BL):
            img = core * BL + b
            sel = ids_keep[img]                               # (75,)
            patchesT[:, KT * b + 1:KT * (b + 1)] = patches[img, sel].T
            posT[:, KT * b] = cls_vec
            posT[:, KT * b + 1:KT * (b + 1)] = pos_full[img, sel].T
            mv[b, 0] = 1.0
            mv[b, 1:] = mask_l[img, np.sort(sel)]
        in_maps.append({
            "patchesT": patchesT,
            "posT": posT.reshape(NCH, 128, T),
            "mvec": mv,
            "wpatchT": wpatchT,
            "wqkvT": wqkvT,
            "wprojT": wprojT,
            "wfc1T": wfc1T,
            "wfc2T": wfc2T,
        })
    return in_maps


_NC_CACHE = {}


def kernel(**inputs):
    if "nc" not in _NC_CACHE:
        _NC_CACHE["nc"] = build()
    nc = _NC_CACHE["nc"]
    in_maps = prep_inputs(inputs)
    res = run_bass_kernel_spmd(nc, in_maps, list(range(NCORES)))
    out = np.concatenate([res.results[i]["out"] for i in range(NCORES)], axis=0)
    return out.astype(np.float32)
